# revision 15
# baseline (speedup 1.0000x reference)
"""Trainium2 Bass kernel for ExpressionAutoDiscretization (embedding_lookup).

Reference computation, per token t (B=8, N=19264, BIN=100, D=768):
    v1 = x_t * w1 + b1                      # (100,)
    v2 = leaky_relu(v1, 0.1)
    v3 = v2 + w2 @ v2 + b2
    w  = softmax(v3)
    e  = w @ emb_table                      # (768,)
    e  = pad_emb.bf16  if pad_mask  else e
    e  = mask_emb.bf16 if masked_mask else e   (mask wins over pad)

Strategy — live-token compaction + f32r matmuls:
  * ~75% of tokens are dead (pad or masked); their outputs are the two
    constant bf16 rows.  The host compacts the live tokens into one dense
    list, splits it evenly over the 8 cores (CAP=4992 slots each — covers
    the 39033 live tokens of the graded input), fills dead rows and
    scatters live rows after the run.  Overflow beyond 8*CAP falls back
    to exact numpy on the host, so correctness holds for any input.
  * v15 pipeline per 512-token supertile (tail tapered to 256+128),
    matmuls in f32r (~13-bit mantissa at full PE streaming rate, fp32
    accumulate — ample for the 2e-2 gate; K=100 >= ~97 keeps full rate):
      Pool: xbb = partition_broadcast of the x row -> SBUF [100, st]
            (src is a slice of the one preloaded x row; the old per-ST
            SWDGE loads + 5-ahead prefetch are gone.  A PE outer product
            was tried instead (v15) and lost 12us: it put the PE on the
            v1 critical path and cost 2 PSUM banks -> ops bufs=2 ->
            mm3 stalled on divides -> PE DVFS clock dropped.)
      ACT:  v2 = Prelu(xbb*w1 + b1, alpha=0.1)  (one op)
      PE:   v3 = (w2^T + I)^T v2            -> PSUM [100, st]
      ACT:  E  = Exp(v3 + b2)               -> SBUF f32r (one op)
      PE:   per 128-token chunk: o = E_chunk^T @ emb_aug [100, 770]
            (col 768 = ones -> softmax denominator)
      DVE:  r = 1/o[:, 768]
      divide+evac (fused): o_s = o[:, 0:768] * r -> bf16 SBUF, chunks
            alternating ACT/DVE.
      Store per 2-chunk pair, as soon as its divides land, alternating
      sync (HWDGE) / gpsimd (SWDGE) rings — the old per-supertile stores
      only started at ~26us and left a 5us DMA drain at the end.
  * All input loads ride ONE scalar-ring DMA each (2 total: a [100, 874]
    const block = emb|w2i|w1,b1,b2 and a [1, 100+CAP] row = w1row|x),
    so the sync ring carries stores only (never interleave loads between
    stores on a HWDGE ring — data corrupts in 8-partition SDMA bands).
  * Output is bf16 (host upcasts): halves the HBM write volume; adds
    <= 2^-9 relative rounding, far inside the error budget.
  * bf16 engine writes must be 4B-aligned slices of a wider SBUF tile;
    dense single-tile bf16 writes hit a broken 2x packing path.
  * PE clock is DVFS-throttled (0.65/1.2/2.4 GHz): it drops after idle
    gaps, so the emission order keeps the PE queue dense.
  * exec time carries ~6us NEFF preamble and semaphore-reset teardown
    that no kernel change can remove.
"""

import numpy as np
import ml_dtypes

BF16 = ml_dtypes.bfloat16
B = 8
N = 19264
BIN = 100
D = 768
EW = D + 2         # emb_aug width: 768 data + denom col + pad col
CAP = 4992         # live-token slots per core (39 chunks of 128)
ST = 512
CH = 128
STS = [128, 384] + [512] * 8 + [256, 128]   # taper head (early first store) and tail
OFFS = [sum(STS[:i]) for i in range(len(STS))]
NST = len(STS)
NSTORE = sum((st // 128 + 1) // 2 for st in STS)

# w1/b1/b2 ride a separate true-f32 [100, 3] block: the BIR verifier
# rejects f32r APs as activation scale operands.
XR = BIN + CAP        # x-row width: w1row | x

_prog_cache = {}


def _build_program(act_div=8):
    import concourse.bacc as bacc
    import concourse.mybir as mybir
    import concourse.tile as tile

    f32 = mybir.dt.float32
    f32r = mybir.dt.float32r
    bf16 = mybir.dt.bfloat16
    AF = mybir.ActivationFunctionType
    Alu = mybir.AluOpType

    nc = bacc.Bacc(
        "TRN2",
        target_bir_lowering=False,
        debug=False,
        enable_asserts=True,
        num_devices=B,
    )

    xr_d = nc.dram_tensor("xr", [1, XR], f32r, kind="ExternalInput")
    xh_d = nc.dram_tensor("xh", [BIN, 128], f32r, kind="ExternalInput")
    w2i_d = nc.dram_tensor("w2i", [BIN, BIN], f32r, kind="ExternalInput")
    emb_d = nc.dram_tensor("emb", [BIN, EW], f32r, kind="ExternalInput")
    cstf_d = nc.dram_tensor("cstf", [BIN, 3], f32, kind="ExternalInput")
    y_d = nc.dram_tensor("y", [CAP, D], bf16, kind="ExternalOutput")

    with tile.TileContext(nc) as tc:
        with (
            tc.tile_pool(name="consts", bufs=1) as consts,
            tc.tile_pool(name="xbp", bufs=4) as xbp,
            tc.tile_pool(name="v2p", bufs=4) as v2p,
            tc.tile_pool(name="ep", bufs=4) as ep,
            tc.tile_pool(name="outs", bufs=5) as outs,
            tc.tile_pool(name="rp", bufs=16) as rp,
            tc.tile_pool(name="v3ps", bufs=2, space="PSUM") as v3ps,
            tc.tile_pool(name="ops", bufs=3, space="PSUM") as ops,
        ):
            xr_t = consts.tile([1, XR], f32r)
            xh_t = consts.tile([BIN, 128], f32r)
            cstf_t = consts.tile([BIN, 3], f32)
            w2i_t = consts.tile([BIN, BIN], f32r)
            emb_t = consts.tile([BIN, EW], f32r)

            # loads split across the two HWDGE rings (sync carries them
            # strictly before any y store rides it).  ST0's xbb comes
            # host-pre-broadcast (xh) so prelu0 need not wait for a
            # gpsimd broadcast.  w2i rides ahead of the big emb block so
            # mm2[0] is not gated on the emb transfer.
            # order matters: each op's data-sem fires only when its whole
            # DMA (16 engine sub-queues) completes, and a ring finishes its
            # DMAs in trigger order — so the op gating the pipeline head
            # (cstf+xh -> prelu0) must ride FIRST with nothing big ahead.
            nc.sync.dma_start(cstf_t[:], cstf_d[:])
            nc.sync.dma_start(xh_t[:], xh_d[:])
            nc.sync.dma_start(xr_t[:], xr_d[:])
            nc.scalar.dma_start(w2i_t[:], w2i_d[:])
            nc.scalar.dma_start(emb_t[:], emb_d[:])

            w1c_t = cstf_t[:, 0:1]
            b1c_t = cstf_t[:, 1:2]
            b2c_t = cstf_t[:, 2:3]

            kdiv = 0
            kst = 0

            def emit_front(k):
                t0, st = OFFS[k], STS[k]
                if k == 0:
                    xbb_t = xh_t
                else:
                    xbb_t = xbp.tile([BIN, ST], f32r, tag="xbb")
                    nc.gpsimd.partition_broadcast(
                        xbb_t[:, 0:st], xr_t[0:1, BIN + t0:BIN + t0 + st],
                    )
                v2_t = v2p.tile([BIN, ST], f32r, tag="v2")
                nc.scalar.activation(
                    v2_t[:, 0:st], xbb_t[:, 0:st], AF.Prelu,
                    bias=b1c_t, scale=w1c_t, alpha=0.1,
                )
                return v2_t

            def emit_mid(v2_t, k):
                st = STS[k]
                v3_p = v3ps.tile([BIN, ST], f32, tag="v3")
                nc.tensor.matmul(
                    v3_p[:, 0:st], w2i_t, v2_t[:, 0:st],
                    start=True, stop=True,
                )
                e_t = ep.tile([BIN, ST], f32r, tag="e")
                nc.scalar.activation(
                    e_t[:, 0:st], v3_p[:, 0:st], AF.Exp, bias=b2c_t,
                )
                return e_t

            def emit_store(o_s, k, c0, nch):
                nonlocal kst
                t0 = OFFS[k] + c0 * CH
                if nch == 1:
                    dstram = y_d[t0:t0 + CH, 0:D]
                    src = o_s[:, c0 * D:(c0 + 1) * D]
                else:
                    dstram = y_d[t0:t0 + nch * CH, 0:D].rearrange(
                        "(c p) d -> p c d", p=CH,
                    )
                    src = o_s[:, c0 * D:(c0 + nch) * D].rearrange(
                        "p (c d) -> p c d", d=D,
                    )
                eng = nc.sync if (kst % 2 == 0 or kst >= NSTORE - 5) else nc.gpsimd
                kst += 1
                eng.dma_start(dstram, src)

            def emit_back(e_t, k):
                nonlocal kdiv
                st = STS[k]
                nch = st // CH
                o_s = outs.tile([CH, 4 * D], bf16, tag="o_s")
                for c in range(nch):
                    cs = c * CH
                    o_p = ops.tile([CH, EW], f32, tag="o_p")
                    nc.tensor.matmul(
                        o_p[:, 0:512], e_t[:, cs:cs + CH],
                        emb_t[:, 0:512], start=True, stop=True,
                    )
                    nc.tensor.matmul(
                        o_p[:, 512:EW], e_t[:, cs:cs + CH],
                        emb_t[:, 512:EW], start=True, stop=True,
                    )
                    r_t = rp.tile([CH, 1], f32, tag="r")
                    # denom = sum of 100 positive exps: no 0/denorm/inf
                    # edge cases, and output is bf16, so the ~18-bit
                    # fast reciprocal is ample.
                    nc.vector.reciprocal_approx_fast(r_t[:], o_p[:, D:D + 1])
                    dst = o_s[:, c * D:(c + 1) * D]
                    pair = kdiv // 2
                    kdiv += 1
                    if (pair * act_div) // 20 != ((pair - 1) * act_div) // 20:
                        nc.scalar.mul(dst, o_p[:, 0:D], r_t[:])
                    else:
                        nc.vector.tensor_scalar(
                            out=dst, in0=o_p[:, 0:D],
                            scalar1=r_t[:], scalar2=None, op0=Alu.mult,
                        )
                    if c % 2 == 1:
                        emit_store(o_s, k, c - 1, 2)
                if nch % 2 == 1:
                    emit_store(o_s, k, nch - 1, 1)

            # software pipeline: front[k+3] | mid[k+2] | back[k]
            fronts = {}
            mids = {}
            for i in range(min(3, NST)):
                fronts[i] = emit_front(i)
            for i in range(min(2, NST)):
                mids[i] = emit_mid(fronts.pop(i), i)
            for k in range(NST):
                if k + 3 < NST:
                    fronts[k + 3] = emit_front(k + 3)
                if k + 2 < NST:
                    mids[k + 2] = emit_mid(fronts.pop(k + 2), k + 2)
                emit_back(mids.pop(k), k)

    nc.compile()
    return nc


def _preprocess(inputs):
    ge = np.ascontiguousarray(np.asarray(inputs["gene_expression"], dtype=np.float32))
    pad = np.asarray(inputs["pad_mask"]) != 0
    msk = np.asarray(inputs["masked_mask"]) != 0
    w1 = np.asarray(inputs["w1"], dtype=np.float32)
    b1 = np.asarray(inputs["b1"], dtype=np.float32)
    w2 = np.asarray(inputs["w2"], dtype=np.float32)
    b2 = np.asarray(inputs["b2"], dtype=np.float32)
    emb = np.asarray(inputs["emb_table"], dtype=np.float32)

    live = ~(pad | msk)
    idx = np.flatnonzero(live.reshape(-1))
    ncap = B * CAP
    idx_dev = idx[:ncap]
    idx_host = idx[ncap:]

    xflat = np.zeros(ncap, np.float32)
    xflat[:len(idx_dev)] = ge.reshape(-1)[idx_dev]
    xcores = xflat.reshape(B, CAP)

    emb_aug = np.zeros((BIN, EW), np.float32)
    emb_aug[:, 0:D] = emb
    emb_aug[:, D] = 1.0
    emb_aug = np.ascontiguousarray(emb_aug)
    w2i = np.ascontiguousarray(w2.T + np.eye(BIN, dtype=np.float32))
    cstf = np.ascontiguousarray(np.stack([w1, b1, b2], axis=1))

    in_maps = []
    for b in range(B):
        xr = np.zeros((1, XR), np.float32)
        xr[0, 0:BIN] = w1
        xr[0, BIN:] = xcores[b]
        xh = np.ascontiguousarray(
            np.broadcast_to(xcores[b][None, 0:128], (BIN, 128)))
        in_maps.append({"w2i": w2i, "emb": emb_aug, "cstf": cstf,
                        "xr": xr, "xh": xh})
    meta = dict(idx_dev=idx_dev, idx_host=idx_host, pad=pad, msk=msk,
                ge=ge, w1=w1, b1=b1, w2=w2, b2=b2, emb=emb,
                pad_emb=np.asarray(inputs["pad_emb"], dtype=np.float32),
                mask_emb=np.asarray(inputs["mask_emb"], dtype=np.float32))
    return in_maps, meta


def _host_tokens(x, w1, b1, w2, b2, emb):
    """Exact reference math for a small set of tokens (overflow fallback)."""
    v1 = x[:, None] * w1[None, :] + b1[None, :]
    v2 = np.where(v1 > 0, v1, 0.1 * v1)
    v3 = v2 + v2 @ w2.T + b2[None, :]
    v3 = v3 - v3.max(axis=1, keepdims=True)
    e = np.exp(v3)
    w = e / e.sum(axis=1, keepdims=True)
    return (w @ emb).astype(np.float32)


def _postprocess(res, meta):
    pad, msk = meta["pad"], meta["msk"]
    out = np.empty((B, N, D), np.float32)
    o2 = out.reshape(-1, D)
    pad_e = meta["pad_emb"].astype(BF16).astype(np.float32)
    mask_e = meta["mask_emb"].astype(BF16).astype(np.float32)
    padonly = (pad & ~msk).reshape(-1)
    o2[padonly] = pad_e
    o2[msk.reshape(-1)] = mask_e
    dev = np.concatenate(
        [np.asarray(res.results[b]["y"]).astype(np.float32) for b in range(B)],
        axis=0,
    )
    idx_dev = meta["idx_dev"]
    o2[idx_dev] = dev[:len(idx_dev)]
    idx_host = meta["idx_host"]
    if len(idx_host):
        xh = meta["ge"].reshape(-1)[idx_host]
        o2[idx_host] = _host_tokens(
            xh, meta["w1"], meta["b1"], meta["w2"], meta["b2"], meta["emb"],
        )
    return out


def _run(inputs, trace=False, trace_cores=None, **kw):
    from concourse.bass_utils import run_bass_kernel_spmd

    key = "v21"
    if key not in _prog_cache:
        _prog_cache[key] = _build_program()
    nc = _prog_cache[key]
    in_maps, meta = _preprocess(inputs)
    res = run_bass_kernel_spmd(
        nc, in_maps, core_ids=list(range(B)),
        trace=trace, trace_cores=trace_cores, **kw,
    )
    out = _postprocess(res, meta)
    return out, res


def kernel(**inputs):
    out, _ = _run(inputs, trace=False)
    return out


# revision 16
# speedup vs baseline: 1.0750x; 1.0750x over previous
"""Trainium2 Bass kernel for ExpressionAutoDiscretization (embedding_lookup).

Reference computation, per token t (B=8, N=19264, BIN=100, D=768):
    v1 = x_t * w1 + b1                      # (100,)
    v2 = leaky_relu(v1, 0.1)
    v3 = v2 + w2 @ v2 + b2
    w  = softmax(v3)
    e  = w @ emb_table                      # (768,)
    e  = pad_emb.bf16  if pad_mask  else e
    e  = mask_emb.bf16 if masked_mask else e   (mask wins over pad)

Strategy — live-token compaction + f32r matmuls:
  * ~75% of tokens are dead (pad or masked); their outputs are the two
    constant bf16 rows.  The host compacts the live tokens into one dense
    list, splits it evenly over the 8 cores (CAP=4992 slots each — covers
    the 39033 live tokens of the graded input), fills dead rows and
    scatters live rows after the run.  Overflow beyond 8*CAP falls back
    to exact numpy on the host, so correctness holds for any input.
  * v15 pipeline per 512-token supertile (tail tapered to 256+128),
    matmuls in f32r (~13-bit mantissa at full PE streaming rate, fp32
    accumulate — ample for the 2e-2 gate; K=100 >= ~97 keeps full rate):
      Pool: xbb = partition_broadcast of the x row -> SBUF [100, st]
            (src is a slice of the one preloaded x row; the old per-ST
            SWDGE loads + 5-ahead prefetch are gone.  A PE outer product
            was tried instead (v15) and lost 12us: it put the PE on the
            v1 critical path and cost 2 PSUM banks -> ops bufs=2 ->
            mm3 stalled on divides -> PE DVFS clock dropped.)
      ACT:  v2 = Prelu(xbb*w1 + b1, alpha=0.1)  (one op)
      PE:   v3 = (w2^T + I)^T v2            -> PSUM [100, st]
      ACT:  E  = Exp(v3 + b2)               -> SBUF f32r (one op)
      PE:   per 128-token chunk: o = E_chunk^T @ emb_aug [100, 770]
            (col 768 = ones -> softmax denominator)
      DVE:  r = 1/o[:, 768]
      divide+evac (fused): o_s = o[:, 0:768] * r -> bf16 SBUF, chunks
            alternating ACT/DVE.
      Store per 2-chunk pair, as soon as its divides land, alternating
      sync (HWDGE) / gpsimd (SWDGE) rings — the old per-supertile stores
      only started at ~26us and left a 5us DMA drain at the end.
  * All input loads ride ONE scalar-ring DMA each (2 total: a [100, 874]
    const block = emb|w2i|w1,b1,b2 and a [1, 100+CAP] row = w1row|x),
    so the sync ring carries stores only (never interleave loads between
    stores on a HWDGE ring — data corrupts in 8-partition SDMA bands).
  * Output is bf16 (host upcasts): halves the HBM write volume; adds
    <= 2^-9 relative rounding, far inside the error budget.
  * bf16 engine writes must be 4B-aligned slices of a wider SBUF tile;
    dense single-tile bf16 writes hit a broken 2x packing path.
  * PE clock is DVFS-throttled (0.65/1.2/2.4 GHz): it drops after idle
    gaps, so the emission order keeps the PE queue dense.
  * exec time carries ~6us NEFF preamble and semaphore-reset teardown
    that no kernel change can remove.
"""

import numpy as np
import ml_dtypes

BF16 = ml_dtypes.bfloat16
B = 8
N = 19264
BIN = 100
D = 768
EW = D + 2         # emb_aug width: 768 data + denom col + pad col
CAP = 4992         # live-token slots per core (39 chunks of 128)
ST = 512
CH = 128
STS = [128, 384] + [512] * 8 + [256, 128]   # taper head (early first store) and tail
OFFS = [sum(STS[:i]) for i in range(len(STS))]
NST = len(STS)
NSTORE = sum((st // 128 + 1) // 2 for st in STS)

# w1/b1/b2 ride a separate true-f32 [100, 3] block: the BIR verifier
# rejects f32r APs as activation scale operands.
XR = BIN + CAP        # x-row width: w1row | x

_prog_cache = {}


def _build_program(act_div=19):
    import concourse.bacc as bacc
    import concourse.mybir as mybir
    import concourse.tile as tile

    f32 = mybir.dt.float32
    f32r = mybir.dt.float32r
    bf16 = mybir.dt.bfloat16
    AF = mybir.ActivationFunctionType
    Alu = mybir.AluOpType

    nc = bacc.Bacc(
        "TRN2",
        target_bir_lowering=False,
        debug=False,
        enable_asserts=True,
        num_devices=B,
    )

    xr_d = nc.dram_tensor("xr", [1, XR], f32r, kind="ExternalInput")
    xh_d = nc.dram_tensor("xh", [BIN, 128], f32r, kind="ExternalInput")
    w2i_d = nc.dram_tensor("w2i", [BIN, BIN], f32r, kind="ExternalInput")
    emb_d = nc.dram_tensor("emb", [BIN, EW], f32r, kind="ExternalInput")
    cstf_d = nc.dram_tensor("cstf", [BIN, 3], f32, kind="ExternalInput")
    y_d = nc.dram_tensor("y", [CAP, D], bf16, kind="ExternalOutput")

    with tile.TileContext(nc) as tc:
        with (
            tc.tile_pool(name="consts", bufs=1) as consts,
            tc.tile_pool(name="xbp", bufs=4) as xbp,
            tc.tile_pool(name="v2p", bufs=4) as v2p,
            tc.tile_pool(name="ep", bufs=4) as ep,
            tc.tile_pool(name="outs", bufs=5) as outs,
            tc.tile_pool(name="rp", bufs=16) as rp,
            tc.tile_pool(name="v3ps", bufs=2, space="PSUM") as v3ps,
            tc.tile_pool(name="ops", bufs=3, space="PSUM") as ops,
        ):
            xr_t = consts.tile([1, XR], f32r)
            xh_t = consts.tile([BIN, 128], f32r)
            cstf_t = consts.tile([BIN, 3], f32)
            w2i_t = consts.tile([BIN, BIN], f32r)
            emb_t = consts.tile([BIN, EW], f32r)

            # loads split across the two HWDGE rings (sync carries them
            # strictly before any y store rides it).  ST0's xbb comes
            # host-pre-broadcast (xh) so prelu0 need not wait for a
            # gpsimd broadcast.  w2i rides ahead of the big emb block so
            # mm2[0] is not gated on the emb transfer.
            # order matters: each op's data-sem fires only when its whole
            # DMA (16 engine sub-queues) completes, and a ring finishes its
            # DMAs in trigger order — so the op gating the pipeline head
            # (cstf+xh -> prelu0) must ride FIRST with nothing big ahead.
            nc.sync.dma_start(cstf_t[:], cstf_d[:])
            nc.sync.dma_start(xh_t[:], xh_d[:])
            nc.sync.dma_start(xr_t[:], xr_d[:])
            nc.scalar.dma_start(w2i_t[:], w2i_d[:])
            nc.scalar.dma_start(emb_t[:], emb_d[:])

            w1c_t = cstf_t[:, 0:1]
            b1c_t = cstf_t[:, 1:2]
            b2c_t = cstf_t[:, 2:3]

            kdiv = 0
            kst = 0

            def emit_front(k):
                t0, st = OFFS[k], STS[k]
                if k == 0:
                    xbb_t = xh_t
                else:
                    xbb_t = xbp.tile([BIN, ST], f32r, tag="xbb")
                    nc.gpsimd.partition_broadcast(
                        xbb_t[:, 0:st], xr_t[0:1, BIN + t0:BIN + t0 + st],
                    )
                v2_t = v2p.tile([BIN, ST], f32r, tag="v2")
                nc.scalar.activation(
                    v2_t[:, 0:st], xbb_t[:, 0:st], AF.Prelu,
                    bias=b1c_t, scale=w1c_t, alpha=0.1,
                )
                return v2_t

            def emit_mid(v2_t, k):
                st = STS[k]
                v3_p = v3ps.tile([BIN, ST], f32, tag="v3")
                nc.tensor.matmul(
                    v3_p[:, 0:st], w2i_t, v2_t[:, 0:st],
                    start=True, stop=True,
                )
                e_t = ep.tile([BIN, ST], f32r, tag="e")
                nc.scalar.activation(
                    e_t[:, 0:st], v3_p[:, 0:st], AF.Exp, bias=b2c_t,
                )
                return e_t

            def emit_store(o_s, k, c0, nch):
                nonlocal kst
                t0 = OFFS[k] + c0 * CH
                if nch == 1:
                    dstram = y_d[t0:t0 + CH, 0:D]
                    src = o_s[:, c0 * D:(c0 + 1) * D]
                else:
                    dstram = y_d[t0:t0 + nch * CH, 0:D].rearrange(
                        "(c p) d -> p c d", p=CH,
                    )
                    src = o_s[:, c0 * D:(c0 + nch) * D].rearrange(
                        "p (c d) -> p c d", d=D,
                    )
                eng = nc.sync if (kst % 2 == 0 or kst >= NSTORE - 5) else nc.gpsimd
                kst += 1
                eng.dma_start(dstram, src)

            def emit_back(e_t, k):
                nonlocal kdiv
                st = STS[k]
                nch = st // CH
                o_s = outs.tile([CH, 4 * D], bf16, tag="o_s")
                for c in range(nch):
                    cs = c * CH
                    o_p = ops.tile([CH, EW], f32, tag="o_p")
                    nc.tensor.matmul(
                        o_p[:, 0:512], e_t[:, cs:cs + CH],
                        emb_t[:, 0:512], start=True, stop=True,
                    )
                    nc.tensor.matmul(
                        o_p[:, 512:EW], e_t[:, cs:cs + CH],
                        emb_t[:, 512:EW], start=True, stop=True,
                    )
                    r_t = rp.tile([CH, 1], f32, tag="r")
                    # denom = sum of 100 positive exps: no 0/denorm/inf
                    # edge cases, and output is bf16, so the ~18-bit
                    # fast reciprocal is ample.
                    nc.vector.reciprocal_approx_fast(r_t[:], o_p[:, D:D + 1])
                    dst = o_s[:, c * D:(c + 1) * D]
                    kdiv += 1
                    if (kdiv * act_div) // 39 != ((kdiv - 1) * act_div) // 39:
                        nc.scalar.mul(dst, o_p[:, 0:D], r_t[:])
                    else:
                        nc.vector.tensor_scalar(
                            out=dst, in0=o_p[:, 0:D],
                            scalar1=r_t[:], scalar2=None, op0=Alu.mult,
                        )
                    if c % 2 == 1:
                        emit_store(o_s, k, c - 1, 2)
                if nch % 2 == 1:
                    emit_store(o_s, k, nch - 1, 1)

            # software pipeline: front[k+3] | mid[k+2] | back[k]
            fronts = {}
            mids = {}
            for i in range(min(3, NST)):
                fronts[i] = emit_front(i)
            for i in range(min(2, NST)):
                mids[i] = emit_mid(fronts.pop(i), i)
            for k in range(NST):
                if k + 3 < NST:
                    fronts[k + 3] = emit_front(k + 3)
                if k + 2 < NST:
                    mids[k + 2] = emit_mid(fronts.pop(k + 2), k + 2)
                emit_back(mids.pop(k), k)

    nc.compile()
    return nc


def _preprocess(inputs):
    ge = np.ascontiguousarray(np.asarray(inputs["gene_expression"], dtype=np.float32))
    pad = np.asarray(inputs["pad_mask"]) != 0
    msk = np.asarray(inputs["masked_mask"]) != 0
    w1 = np.asarray(inputs["w1"], dtype=np.float32)
    b1 = np.asarray(inputs["b1"], dtype=np.float32)
    w2 = np.asarray(inputs["w2"], dtype=np.float32)
    b2 = np.asarray(inputs["b2"], dtype=np.float32)
    emb = np.asarray(inputs["emb_table"], dtype=np.float32)

    live = ~(pad | msk)
    idx = np.flatnonzero(live.reshape(-1))
    ncap = B * CAP
    idx_dev = idx[:ncap]
    idx_host = idx[ncap:]

    xflat = np.zeros(ncap, np.float32)
    xflat[:len(idx_dev)] = ge.reshape(-1)[idx_dev]
    xcores = xflat.reshape(B, CAP)

    emb_aug = np.zeros((BIN, EW), np.float32)
    emb_aug[:, 0:D] = emb
    emb_aug[:, D] = 1.0
    emb_aug = np.ascontiguousarray(emb_aug)
    w2i = np.ascontiguousarray(w2.T + np.eye(BIN, dtype=np.float32))
    cstf = np.ascontiguousarray(np.stack([w1, b1, b2], axis=1))

    in_maps = []
    for b in range(B):
        xr = np.zeros((1, XR), np.float32)
        xr[0, 0:BIN] = w1
        xr[0, BIN:] = xcores[b]
        xh = np.ascontiguousarray(
            np.broadcast_to(xcores[b][None, 0:128], (BIN, 128)))
        in_maps.append({"w2i": w2i, "emb": emb_aug, "cstf": cstf,
                        "xr": xr, "xh": xh})
    meta = dict(idx_dev=idx_dev, idx_host=idx_host, pad=pad, msk=msk,
                ge=ge, w1=w1, b1=b1, w2=w2, b2=b2, emb=emb,
                pad_emb=np.asarray(inputs["pad_emb"], dtype=np.float32),
                mask_emb=np.asarray(inputs["mask_emb"], dtype=np.float32))
    return in_maps, meta


def _host_tokens(x, w1, b1, w2, b2, emb):
    """Exact reference math for a small set of tokens (overflow fallback)."""
    v1 = x[:, None] * w1[None, :] + b1[None, :]
    v2 = np.where(v1 > 0, v1, 0.1 * v1)
    v3 = v2 + v2 @ w2.T + b2[None, :]
    v3 = v3 - v3.max(axis=1, keepdims=True)
    e = np.exp(v3)
    w = e / e.sum(axis=1, keepdims=True)
    return (w @ emb).astype(np.float32)


def _postprocess(res, meta):
    pad, msk = meta["pad"], meta["msk"]
    out = np.empty((B, N, D), np.float32)
    o2 = out.reshape(-1, D)
    pad_e = meta["pad_emb"].astype(BF16).astype(np.float32)
    mask_e = meta["mask_emb"].astype(BF16).astype(np.float32)
    padonly = (pad & ~msk).reshape(-1)
    o2[padonly] = pad_e
    o2[msk.reshape(-1)] = mask_e
    dev = np.concatenate(
        [np.asarray(res.results[b]["y"]).astype(np.float32) for b in range(B)],
        axis=0,
    )
    idx_dev = meta["idx_dev"]
    o2[idx_dev] = dev[:len(idx_dev)]
    idx_host = meta["idx_host"]
    if len(idx_host):
        xh = meta["ge"].reshape(-1)[idx_host]
        o2[idx_host] = _host_tokens(
            xh, meta["w1"], meta["b1"], meta["w2"], meta["b2"], meta["emb"],
        )
    return out


def _run(inputs, trace=False, trace_cores=None, **kw):
    from concourse.bass_utils import run_bass_kernel_spmd

    key = "v22"
    if key not in _prog_cache:
        _prog_cache[key] = _build_program()
    nc = _prog_cache[key]
    in_maps, meta = _preprocess(inputs)
    res = run_bass_kernel_spmd(
        nc, in_maps, core_ids=list(range(B)),
        trace=trace, trace_cores=trace_cores, **kw,
    )
    out = _postprocess(res, meta)
    return out, res


def kernel(**inputs):
    out, _ = _run(inputs, trace=False)
    return out
